# revision 10
# baseline (speedup 1.0000x reference)
"""Trainium2 Bass kernel for nn_DiffusionNet (8 NeuronCores).

Sharding: 2 batch groups x 4 cores.
  Seq stack (24 layer-apps): TP over heads/hidden, bf16 AllReduce after wo/ff2.
  Atom stack (10 layers): token-parallel (512/core), replicated K/V, AllGather.
Activations kept feature-major ("transposed" [feat, tok]); every GEMM is
lhsT=weight-chunk [128,128] x rhs=activation [128,512]. Matmul I/O bf16,
PSUM/residual f32.
"""
import sys
sys.path.insert(0, '/opt/trn_rl_repo')
import numpy as np
import ml_dtypes

BF16 = ml_dtypes.bfloat16

D, A, H, DK = 1024, 256, 16, 64
L, NA, B = 512, 2048, 2
SEQ_DEPTH, CYCLES, ATOM_DEPTH = 8, 3, 10
EPS = 1e-5
SIGDATA, VARDATA = 16.0, 256.0
N_CORES = 8
GROUPS = [[0, 1, 2, 3], [4, 5, 6, 7]]

_BUILT = None


# ---------------------------------------------------------------- host math
def _ln_np(x, g, b):
    m = x.mean(-1, keepdims=True)
    v = x.var(-1, keepdims=True)
    return (x - m) / np.sqrt(v + EPS) * g + b


def _alibi_bias():
    def pow2(n):
        start = 2.0 ** (-2.0 ** (-(np.log2(n) - 3)))
        return [start * start ** i for i in range(n)]
    if np.log2(H).is_integer():
        s = pow2(H)
    else:
        c = 2 ** int(np.floor(np.log2(H)))
        s = pow2(c) + pow2(2 * c)[0::2][: H - c]
    slopes = np.asarray(s, dtype=np.float32)
    idx = np.arange(L)
    rel = np.abs(idx[None, :] - idx[:, None]).astype(np.float32)
    return -slopes[:, None, None] * rel[None]  # [H,L,L]


def _pos_encoding(positions, d):
    pos = positions.astype(np.float32)[:, None]
    div = np.exp(np.arange(0, d, 2, dtype=np.float32) * (-np.log(10000.0) / d))
    ang = pos * div
    return np.stack([np.sin(ang), np.cos(ang)], -1).reshape(positions.shape[0], d)


def _fourier_embed(nlev, p):
    scales = 2.0 ** np.arange(8, dtype=np.float32)
    xs = nlev[:, None] / scales
    f = np.concatenate([np.sin(xs), np.cos(xs)], axis=-1).astype(np.float32)
    return _ln_np(f @ p['w'] + p['b'], p['g'], p['beta'])


# ---------------------------------------------------------------- builder
def _build():
    import concourse.bacc as bacc
    import concourse.mybir as mybir
    import concourse.tile as tile
    from contextlib import ExitStack

    dt = mybir.dt
    AF = mybir.ActivationFunctionType
    OP = mybir.AluOpType
    f32, bf16 = dt.float32, dt.bfloat16
    nc = bacc.Bacc("TRN2", target_bir_lowering=False, debug=False,
                   num_devices=N_CORES)

    def din(name, shape, d=bf16):
        return nc.dram_tensor(name, list(shape), d, kind="ExternalInput")

    xT0 = din("xT0", [8, 128, L], f32)
    s_wqkv = din("s_wqkv", [SEQ_DEPTH, 8, 128, 768])
    s_wo = din("s_wo", [SEQ_DEPTH, 2, 128, D])
    s_gw = din("s_gw", [SEQ_DEPTH, 8, 128, D])
    s_gb = din("s_gb", [SEQ_DEPTH, 128, 8], f32)
    s_ln = din("s_ln", [SEQ_DEPTH, 4, 128, 8], f32)
    s_ff1 = din("s_ff1", [SEQ_DEPTH, 8, 128, 2048])
    s_ff2 = din("s_ff2", [SEQ_DEPTH, 8, 128, D])
    expb_d = din("expb", [4, 4, 128, L])
    w4_d = din("w4", [8, 128, 4], f32)
    toat_d = din("to_atom", [8, 128, A], f32)
    norm2_d = din("norm2", [2, 1, A], f32)
    norm3_d = din("norm3", [2, 128, 2], f32)
    gloc_d = din("g_loc", [4, 128, L])
    embT_d = din("embedT", [2, 128, L], f32)
    a_wqkv = din("a_wqkv", [ATOM_DEPTH, 2, 128, 768])
    a_gw = din("a_gw", [ATOM_DEPTH, 2, 128, A])
    a_gb = din("a_gb", [ATOM_DEPTH, 128, 2], f32)
    a_lnw = din("a_ln", [ATOM_DEPTH, 4, 128, 2], f32)
    a_ff1 = din("a_ff1", [ATOM_DEPTH, 2, 128, 1024])
    a_ff1b = din("a_ff1b", [ATOM_DEPTH, 128, 8], f32)
    a_ff2 = din("a_ff2", [ATOM_DEPTH, 4, 128, A])
    a_ff2b = din("a_ff2b", [ATOM_DEPTH, 128, 2], f32)
    outln_d = din("out_lnw", [2, 128, 2], f32)
    outw_d = din("out_w", [2, 128, 4])

    coords4 = nc.dram_tensor("coords4", [4, 128, 4], f32, kind="ExternalOutput")
    denT_d = nc.dram_tensor("denT", [4, L], f32, kind="ExternalOutput")

    with tile.TileContext(nc) as tc:
        es = ExitStack()
        pconst = es.enter_context(tc.tile_pool(name="pconst", bufs=1))
        pact = es.enter_context(tc.tile_pool(name="pact", bufs=1))
        pscr = es.enter_context(tc.tile_pool(name="pscr", bufs=2))
        pstat = es.enter_context(tc.tile_pool(name="pstat", bufs=2))
        ppmm = es.enter_context(tc.tile_pool(name="ppmm", bufs=2, space="PSUM"))
        pprow = es.enter_context(tc.tile_pool(name="pprow", bufs=2, space="PSUM"))
        ppbc = es.enter_context(tc.tile_pool(name="ppbc", bufs=2, space="PSUM"))
        ppo = es.enter_context(tc.tile_pool(name="ppo", bufs=2, space="PSUM"))
        pdram = es.enter_context(tc.tile_pool(name="pdram", bufs=1, space="DRAM"))

        ones_col = pconst.tile([128, 1], bf16)
        nc.vector.memset(ones_col[:], 1.0)
        ones_row = pconst.tile([1, 128], f32)
        nc.vector.memset(ones_row[:], 1.0)
        eps_col = pconst.tile([128, 1], f32)
        nc.vector.memset(eps_col[:], EPS)

        xT = pact.tile([128, 8 * L], f32, name="xT")
        hnT = pact.tile([128, 8 * L], bf16, name="hnT")
        hnloc = pact.tile([128, 2 * L], bf16, name="hnloc")
        a_nat = pact.tile([128, 4 * A], bf16, name="a_nat")
        emb_t = pact.tile([128, 2 * L], f32, name="emb_t")
        xloc = pact.tile([128, 2 * L], f32, name="xloc")
        obuf = pact.tile([128, 8 * L], bf16, name="obuf")

        bnc_in = pdram.tile([128, 8 * L], bf16, name="bnc_in")
        bnc_out = pdram.tile([128, 8 * L], bf16, name="bnc_out")
        ag_in = pdram.tile([128, 2 * L], f32, name="ag_in")
        ag_out = pdram.tile([4, 128, 2 * L], f32, name="ag_out")

        for c in range(8):
            nc.sync.dma_start(xT[:, c * L:(c + 1) * L], xT0[c])

        # ---------------------------------------------- helpers
        def ln_partition(src_tile, src_stride, tok_off, T, nchunks, g_col,
                         b_col, dst_tile, dst_stride, dst_off, inv_dim):
            """LN over partition axis (features = nchunks*128) for T tokens.
            src chunk c: src_tile[:, c*src_stride + tok_off : +T] (f32).
            dst chunk c: dst_tile[:, c*dst_stride + dst_off : +T] (bf16)."""
            s_ps = pprow.tile([1, T], f32, tag="lnrow")
            ss_ps = pprow.tile([1, T], f32, tag="lnrow")
            for c in range(nchunks):
                sl = src_tile[:, c * src_stride + tok_off:
                              c * src_stride + tok_off + T]
                xb = pscr.tile([128, T], bf16, tag="lnxb")
                nc.vector.tensor_copy(xb[:], sl)
                sq = pscr.tile([128, T], bf16, tag="lnsq")
                nc.vector.tensor_mul(sq[:], xb[:], xb[:])
                nc.tensor.matmul(s_ps[:], ones_col[:], xb[:],
                                 start=(c == 0), stop=(c == nchunks - 1))
                nc.tensor.matmul(ss_ps[:], ones_col[:], sq[:],
                                 start=(c == 0), stop=(c == nchunks - 1))
            mean = pstat.tile([1, T], f32, tag="mean")
            nc.vector.tensor_scalar_mul(mean[:], s_ps[:], inv_dim)
            tmp = pstat.tile([1, T], f32, tag="stmp")
            nc.vector.tensor_scalar_mul(tmp[:], ss_ps[:], inv_dim)
            m2 = pstat.tile([1, T], f32, tag="sm2")
            nc.vector.tensor_mul(m2[:], mean[:], mean[:])
            nc.vector.tensor_sub(tmp[:], tmp[:], m2[:])
            nc.scalar.activation(tmp[:], tmp[:], AF.Sqrt,
                                 bias=eps_col[:1, 0:1])
            rstd = pstat.tile([1, T], f32, tag="rstd")
            nc.vector.reciprocal(rstd[:], tmp[:])
            bm = ppbc.tile([128, T], f32, tag="bcast")
            nc.tensor.matmul(bm[:], ones_row[:], mean[:], start=True, stop=True)
            br = ppbc.tile([128, T], f32, tag="bcast")
            nc.tensor.matmul(br[:], ones_row[:], rstd[:], start=True, stop=True)
            for c in range(nchunks):
                sl = src_tile[:, c * src_stride + tok_off:
                              c * src_stride + tok_off + T]
                t1 = pscr.tile([128, T], f32, tag="lnt1")
                nc.vector.tensor_sub(t1[:], sl, bm[:])
                t2 = pscr.tile([128, T], f32, tag="lnt2")
                nc.vector.tensor_mul(t2[:], t1[:], br[:])
                nc.vector.tensor_scalar(
                    dst_tile[:, c * dst_stride + dst_off:
                             c * dst_stride + dst_off + T],
                    t2[:], g_col[:, c:c + 1], b_col[:, c:c + 1],
                    op0=OP.mult, op1=OP.add)

        # ================================================== SEQ PHASE
        with ExitStack() as seq_es:
            pseq = seq_es.enter_context(tc.tile_pool(name="pseq", bufs=1))
            pw_qkv = seq_es.enter_context(tc.tile_pool(name="pw_qkv", bufs=1))
            pw_wo = seq_es.enter_context(tc.tile_pool(name="pw_wo", bufs=1))
            pw_gw = seq_es.enter_context(tc.tile_pool(name="pw_gw", bufs=1))
            pw_ffq = seq_es.enter_context(tc.tile_pool(name="pw_ffq", bufs=2))
            pw_ln = seq_es.enter_context(tc.tile_pool(name="pw_ln", bufs=2))

            qT = pseq.tile([128, 2 * L], bf16, name="qT")
            kT = pseq.tile([128, 2 * L], bf16, name="kT")
            vN = pseq.tile([128, 4 * A], bf16, name="vN")
            oT = pseq.tile([128, 2 * L], bf16, name="oT")
            swiT = pseq.tile([128, 8 * L], bf16, name="swiT")
            expb_sb = pseq.tile([128, 16 * L], bf16, name="expb_sb")
            for h in range(4):
                for kvc in range(4):
                    i = h * 4 + kvc
                    nc.sync.dma_start(expb_sb[:, i * L:(i + 1) * L],
                                      expb_d[h, kvc])

            for visit in range(CYCLES * SEQ_DEPTH):
                lyr = visit % SEQ_DEPTH
                wqkv = pw_qkv.tile([128, 8 * 768], bf16, tag="qkv")
                for c in range(8):
                    nc.sync.dma_start(wqkv[:, c * 768:(c + 1) * 768],
                                      s_wqkv[lyr, c])
                lnw = pw_ln.tile([128, 4 * 8], f32, tag="ln")
                for i in range(4):
                    nc.sync.dma_start(lnw[:, i * 8:(i + 1) * 8], s_ln[lyr, i])
                gbc = pw_ln.tile([128, 8], f32, tag="gb")
                nc.sync.dma_start(gbc[:], s_gb[lyr])

                # ln1 -> hnT
                ln_partition(xT, L, 0, L, 8, lnw[:, 0:8], lnw[:, 8:16],
                             hnT, L, 0, 1.0 / D)

                # q^T/k^T head-pair packs; v natural
                for pair in range(2):
                    for which, dstt in ((0, qT), (2, kT)):
                        ps = ppmm.tile([128, L], f32, tag="mm")
                        for cc in range(8):
                            co = cc * 768 + (which + pair) * 128
                            nc.tensor.matmul(ps[:], wqkv[:, co:co + 128],
                                             hnT[:, cc * L:(cc + 1) * L],
                                             start=(cc == 0), stop=(cc == 7))
                        nc.vector.tensor_copy(
                            dstt[:, pair * L:(pair + 1) * L], ps[:])
                for tci in range(4):
                    ps = ppmm.tile([128, A], f32, tag="mm")
                    for cc in range(8):
                        nc.tensor.matmul(
                            ps[:],
                            hnT[:, cc * L + tci * 128: cc * L + tci * 128 + 128],
                            wqkv[:, cc * 768 + 512: cc * 768 + 768],
                            start=(cc == 0), stop=(cc == 7))
                    nc.vector.tensor_copy(vN[:, tci * A:(tci + 1) * A], ps[:])

                wo_t = pw_wo.tile([128, 2 * D], bf16, tag="wo")
                for c in range(2):
                    nc.sync.dma_start(wo_t[:, c * D:(c + 1) * D], s_wo[lyr, c])

                # attention per local head
                for h in range(4):
                    po = (h % 2) * 64
                    cb = (h // 2) * L
                    sum_ps = pprow.tile([1, L], f32, tag="lnrow")
                    oacc = ppo.tile([64, L], f32, tag="oacc")
                    for kvc in range(4):
                        ps = ppmm.tile([128, L], f32, tag="mm")
                        nc.tensor.matmul(
                            ps[:],
                            kT[po:po + 64, cb + kvc * 128: cb + kvc * 128 + 128],
                            qT[po:po + 64, cb: cb + L], start=True, stop=True)
                        e1 = pscr.tile([128, L], bf16, tag="es1")
                        nc.scalar.activation(e1[:], ps[:], AF.Exp)
                        e2 = pscr.tile([128, L], bf16, tag="es2")
                        i = h * 4 + kvc
                        nc.vector.tensor_mul(e2[:], e1[:],
                                             expb_sb[:, i * L:(i + 1) * L])
                        nc.tensor.matmul(sum_ps[:], ones_col[:], e2[:],
                                         start=(kvc == 0), stop=(kvc == 3))
                        nc.tensor.matmul(
                            oacc[:], vN[:, kvc * A + h * 64: kvc * A + h * 64 + 64],
                            e2[:], start=(kvc == 0), stop=(kvc == 3))
                    rinv = pstat.tile([1, L], f32, tag="rinv")
                    nc.vector.reciprocal(rinv[:], sum_ps[:])
                    brs = ppbc.tile([64, L], f32, tag="bcast")
                    nc.tensor.matmul(brs[:], ones_row[:, :64], rinv[:],
                                     start=True, stop=True)
                    brv = pscr.tile([64, L], f32, tag="brv")
                    nc.vector.tensor_copy(brv[:], brs[:])
                    nc.vector.tensor_mul(oT[po:po + 64, cb:cb + L], oacc[:],
                                         brv[:])

                # wo partial -> obuf (bf16), AllReduce
                for mc in range(8):
                    ps = ppmm.tile([128, L], f32, tag="mm")
                    for cc in range(2):
                        nc.tensor.matmul(
                            ps[:], wo_t[:, cc * D + mc * 128: cc * D + mc * 128 + 128],
                            oT[:, cc * L:(cc + 1) * L],
                            start=(cc == 0), stop=(cc == 1))
                    nc.vector.tensor_copy(obuf[:, mc * L:(mc + 1) * L], ps[:])

                gw_t = pw_gw.tile([128, 8 * D], bf16, tag="gw")
                for c in range(8):
                    nc.sync.dma_start(gw_t[:, c * D:(c + 1) * D], s_gw[lyr, c])

                nc.sync.dma_start(bnc_in[:], obuf[:])
                nc.gpsimd.collective_compute(
                    "AllReduce", OP.add, replica_groups=GROUPS,
                    ins=[bnc_in[:]], outs=[bnc_out[:]])
                nc.sync.dma_start(obuf[:], bnc_out[:])

                # gate & residual
                for mc in range(8):
                    ps = ppmm.tile([128, L], f32, tag="mm")
                    for cc in range(8):
                        nc.tensor.matmul(
                            ps[:], gw_t[:, cc * D + mc * 128: cc * D + mc * 128 + 128],
                            hnT[:, cc * L:(cc + 1) * L],
                            start=(cc == 0), stop=(cc == 7))
                    sg = pscr.tile([128, L], bf16, tag="sg")
                    nc.scalar.activation(sg[:], ps[:], AF.Sigmoid,
                                         bias=gbc[:, mc:mc + 1])
                    t = pscr.tile([128, L], f32, tag="gt")
                    nc.vector.tensor_mul(t[:], sg[:],
                                         obuf[:, mc * L:(mc + 1) * L])
                    nc.vector.tensor_add(xT[:, mc * L:(mc + 1) * L],
                                         xT[:, mc * L:(mc + 1) * L], t[:])

                # ln2 -> hnT
                ln_partition(xT, L, 0, L, 8, lnw[:, 16:24], lnw[:, 24:32],
                             hnT, L, 0, 1.0 / D)

                # ff1 in quarters (a: q 0-3 -> swiT; g: silu * into swiT)
                for half, base in ((0, 0), (1, 1024)):
                    for q in range(2):
                        ffq = pw_ffq.tile([128, 8 * 512], bf16, tag="ffq")
                        for cc in range(8):
                            src = s_ff1[lyr, cc][:, base + q * 512:
                                                 base + (q + 1) * 512]
                            nc.sync.dma_start(
                                ffq[:, cc * 512:(cc + 1) * 512], src)
                        for j in range(4):
                            mc = q * 4 + j
                            ps = ppmm.tile([128, L], f32, tag="mm")
                            for cc in range(8):
                                nc.tensor.matmul(
                                    ps[:],
                                    ffq[:, cc * 512 + j * 128: cc * 512 + j * 128 + 128],
                                    hnT[:, cc * L:(cc + 1) * L],
                                    start=(cc == 0), stop=(cc == 7))
                            if half == 0:
                                nc.vector.tensor_copy(
                                    swiT[:, mc * L:(mc + 1) * L], ps[:])
                            else:
                                sg = pscr.tile([128, L], bf16, tag="silu")
                                nc.scalar.activation(sg[:], ps[:], AF.Silu)
                                nc.vector.tensor_mul(
                                    swiT[:, mc * L:(mc + 1) * L],
                                    swiT[:, mc * L:(mc + 1) * L], sg[:])

                for hh in range(2):
                    ff2h = pw_ffq.tile([128, 8 * 512], bf16, tag="ffq")
                    for cc in range(8):
                        nc.sync.dma_start(
                            ff2h[:, cc * 512:(cc + 1) * 512],
                            s_ff2[lyr, cc][:, hh * 512:(hh + 1) * 512])
                    for j in range(4):
                        mc = hh * 4 + j
                        ps = ppmm.tile([128, L], f32, tag="mm")
                        for cc in range(8):
                            nc.tensor.matmul(
                                ps[:],
                                ff2h[:, cc * 512 + j * 128: cc * 512 + j * 128 + 128],
                                swiT[:, cc * L:(cc + 1) * L],
                                start=(cc == 0), stop=(cc == 7))
                        nc.vector.tensor_copy(obuf[:, mc * L:(mc + 1) * L],
                                              ps[:])
                nc.sync.dma_start(bnc_in[:], obuf[:])
                nc.gpsimd.collective_compute(
                    "AllReduce", OP.add, replica_groups=GROUPS,
                    ins=[bnc_in[:]], outs=[bnc_out[:]])
                nc.sync.dma_start(obuf[:], bnc_out[:])
                for mc in range(8):
                    nc.vector.tensor_add(xT[:, mc * L:(mc + 1) * L],
                                         xT[:, mc * L:(mc + 1) * L],
                                         obuf[:, mc * L:(mc + 1) * L])

            # ---------------- heads + embed (still in seq scope) ----------
            w4_t = pconst.tile([128, 8 * 4], f32)
            for c in range(8):
                nc.sync.dma_start(w4_t[:, c * 4:(c + 1) * 4], w4_d[c])
            for tci in range(4):
                ps = ppmm.tile([128, 4], f32, tag="mm")
                for cc in range(8):
                    nc.tensor.matmul(
                        ps[:], xT[:, cc * L + tci * 128: cc * L + tci * 128 + 128],
                        w4_t[:, cc * 4:(cc + 1) * 4],
                        start=(cc == 0), stop=(cc == 7))
                ot = pscr.tile([128, 4], f32, tag="c4")
                nc.vector.tensor_copy(ot[:], ps[:])
                nc.sync.dma_start(coords4[tci], ot[:])

            toat_t = pconst.tile([128, 8 * A], f32)
            for c in range(8):
                nc.sync.dma_start(toat_t[:, c * A:(c + 1) * A], toat_d[c])
            n2g = pconst.tile([1, A], f32)
            nc.sync.dma_start(n2g[:], norm2_d[0])
            n2b = pconst.tile([1, A], f32)
            nc.sync.dma_start(n2b[:], norm2_d[1])
            n2g_sb = pconst.tile([128, A], f32)
            n2b_sb = pconst.tile([128, A], f32)
            bc = ppbc.tile([128, A], f32, tag="bcast")
            nc.tensor.matmul(bc[:], ones_row[:], n2g[:], start=True, stop=True)
            nc.vector.tensor_copy(n2g_sb[:], bc[:])
            bc2 = ppbc.tile([128, A], f32, tag="bcast")
            nc.tensor.matmul(bc2[:], ones_row[:], n2b[:], start=True, stop=True)
            nc.vector.tensor_copy(n2b_sb[:], bc2[:])

            for tci in range(4):
                ps = ppmm.tile([128, A], f32, tag="mm")
                for cc in range(8):
                    nc.tensor.matmul(
                        ps[:], xT[:, cc * L + tci * 128: cc * L + tci * 128 + 128],
                        toat_t[:, cc * A:(cc + 1) * A],
                        start=(cc == 0), stop=(cc == 7))
                s_r = pstat.tile([128, 1], f32, tag="nr")
                nc.vector.reduce_sum(s_r[:], ps[:], axis=mybir.AxisListType.X)
                mean = pstat.tile([128, 1], f32, tag="nm")
                nc.vector.tensor_scalar_mul(mean[:], s_r[:], 1.0 / A)
                sq = pscr.tile([128, A], f32, tag="nsq")
                nc.scalar.activation(sq[:], ps[:], AF.Square)
                ss_r = pstat.tile([128, 1], f32, tag="nss")
                nc.vector.reduce_sum(ss_r[:], sq[:], axis=mybir.AxisListType.X)
                ex2 = pstat.tile([128, 1], f32, tag="ne2")
                nc.vector.tensor_scalar_mul(ex2[:], ss_r[:], 1.0 / A)
                m2 = pstat.tile([128, 1], f32, tag="nm2")
                nc.vector.tensor_mul(m2[:], mean[:], mean[:])
                nc.vector.tensor_sub(ex2[:], ex2[:], m2[:])
                nc.scalar.activation(ex2[:], ex2[:], AF.Sqrt,
                                     bias=eps_col[:])
                rstd = pstat.tile([128, 1], f32, tag="nrs")
                nc.vector.reciprocal(rstd[:], ex2[:])
                t1 = pscr.tile([128, A], f32, tag="nt1")
                nc.vector.tensor_scalar(t1[:], ps[:], mean[:], rstd[:],
                                        op0=OP.subtract, op1=OP.mult)
                t2 = pscr.tile([128, A], f32, tag="nt2")
                nc.vector.tensor_mul(t2[:], t1[:], n2g_sb[:])
                nc.vector.tensor_add(a_nat[:, tci * A:(tci + 1) * A], t2[:],
                                     n2b_sb[:])

            gl_t = pconst.tile([128, 4 * L], bf16)
            for c in range(4):
                nc.sync.dma_start(gl_t[:, c * L:(c + 1) * L], gloc_d[c])
            for c in range(2):
                nc.sync.dma_start(emb_t[:, c * L:(c + 1) * L], embT_d[c])
            n3 = pconst.tile([128, 4], f32)
            nc.sync.dma_start(n3[:, 0:2], norm3_d[0])
            nc.sync.dma_start(n3[:, 2:4], norm3_d[1])

            for mc in range(2):
                ps = ppmm.tile([128, L], f32, tag="mm")
                for cc in range(4):
                    nc.tensor.matmul(
                        ps[:], a_nat[:, cc * A + mc * 128: cc * A + mc * 128 + 128],
                        gl_t[:, cc * L:(cc + 1) * L],
                        start=(cc == 0), stop=(cc == 3))
                nc.vector.tensor_add(xloc[:, mc * L:(mc + 1) * L], ps[:],
                                     emb_t[:, mc * L:(mc + 1) * L])
            ln_partition(xloc, L, 0, L, 2, n3[:, 0:2], n3[:, 2:4],
                         hnloc, L, 0, 1.0 / A)
            for mc in range(2):
                nc.vector.tensor_copy(xloc[:, mc * L:(mc + 1) * L],
                                      hnloc[:, mc * L:(mc + 1) * L])

        # ================================================== ATOM PHASE
        patom = es.enter_context(tc.tile_pool(name="patom", bufs=1))
        paw = es.enter_context(tc.tile_pool(name="paw", bufs=2))
        qTa = patom.tile([128, 2 * L], bf16, name="qTa")     # also reused as oTa
        kTa = patom.tile([128, 2 * 2048], bf16, name="kTa")
        vNa = patom.tile([128, 16 * A], bf16, name="vNa")
        swiA = patom.tile([128, 4 * L], bf16, name="swiA")

        def atom_allgather():
            nc.sync.dma_start(ag_in[:], xloc[:])
            nc.gpsimd.collective_compute(
                "AllGather", OP.bypass, replica_groups=GROUPS,
                ins=[ag_in[:]], outs=[ag_out[:]])
            nc.sync.dma_start(
                xT[:, 0:4096].rearrange("p (c r t) -> p c r t", c=2, r=4),
                ag_out[:].rearrange("r p (c t) -> p c r t", c=2))

        atom_allgather()

        for lyr in range(ATOM_DEPTH):
            awqkv = paw.tile([128, 2 * 768], bf16, tag="aqkv")
            for c in range(2):
                nc.sync.dma_start(awqkv[:, c * 768:(c + 1) * 768],
                                  a_wqkv[lyr, c])
            alnw = paw.tile([128, 4 * 2], f32, tag="aln")
            for i in range(4):
                nc.sync.dma_start(alnw[:, i * 2:(i + 1) * 2], a_lnw[lyr, i])
            agw_t = paw.tile([128, 2 * A], bf16, tag="agw")
            for c in range(2):
                nc.sync.dma_start(agw_t[:, c * A:(c + 1) * A], a_gw[lyr, c])
            agb_t = paw.tile([128, 2], f32, tag="agb")
            nc.sync.dma_start(agb_t[:], a_gb[lyr])

            # ln1 full (4 token blocks) -> hnT [128, c*2048 + t]
            for tb in range(4):
                ln_partition(xT, 2048, tb * L, L, 2, alnw[:, 0:2],
                             alnw[:, 2:4], hnT, 2048, tb * L, 1.0 / A)
            # ln1 local -> hnloc
            ln_partition(xloc, L, 0, L, 2, alnw[:, 0:2], alnw[:, 2:4],
                         hnloc, L, 0, 1.0 / A)

            # qT local [A,512]
            for mc in range(2):
                ps = ppmm.tile([128, L], f32, tag="mm")
                for cc in range(2):
                    nc.tensor.matmul(
                        ps[:], awqkv[:, cc * 768 + mc * 128: cc * 768 + mc * 128 + 128],
                        hnloc[:, cc * L:(cc + 1) * L],
                        start=(cc == 0), stop=(cc == 1))
                nc.vector.tensor_copy(qTa[:, mc * L:(mc + 1) * L], ps[:])
            # kT full [A,2048]
            for mc in range(2):
                for tb in range(4):
                    ps = ppmm.tile([128, L], f32, tag="mm")
                    for cc in range(2):
                        nc.tensor.matmul(
                            ps[:],
                            awqkv[:, cc * 768 + 256 + mc * 128: cc * 768 + 256 + mc * 128 + 128],
                            hnT[:, cc * 2048 + tb * L: cc * 2048 + (tb + 1) * L],
                            start=(cc == 0), stop=(cc == 1))
                    nc.vector.tensor_copy(
                        kTa[:, mc * 2048 + tb * L: mc * 2048 + (tb + 1) * L],
                        ps[:])
            # v natural full [2048, A]
            for tkc in range(16):
                ps = ppmm.tile([128, A], f32, tag="mm")
                for cc in range(2):
                    nc.tensor.matmul(
                        ps[:],
                        hnT[:, cc * 2048 + tkc * 128: cc * 2048 + tkc * 128 + 128],
                        awqkv[:, cc * 768 + 512: cc * 768 + 768],
                        start=(cc == 0), stop=(cc == 1))
                nc.vector.tensor_copy(vNa[:, tkc * A:(tkc + 1) * A], ps[:])

            # attention: scores^T chunks, exp, sum, o' accumulation
            sum_ps = pprow.tile([1, L], f32, tag="lnrow")
            oacc0 = ppo.tile([128, L], f32, tag="oacc")
            oacc1 = ppo.tile([128, L], f32, tag="oacc")
            for kvc in range(16):
                ps = ppmm.tile([128, L], f32, tag="mm")
                for cc in range(2):
                    nc.tensor.matmul(
                        ps[:], kTa[:, cc * 2048 + kvc * 128: cc * 2048 + kvc * 128 + 128],
                        qTa[:, cc * L:(cc + 1) * L],
                        start=(cc == 0), stop=(cc == 1))
                e1 = pscr.tile([128, L], bf16, tag="es1")
                nc.scalar.activation(e1[:], ps[:], AF.Exp)
                nc.tensor.matmul(sum_ps[:], ones_col[:], e1[:],
                                 start=(kvc == 0), stop=(kvc == 15))
                nc.tensor.matmul(oacc0[:], vNa[:, kvc * A: kvc * A + 128],
                                 e1[:], start=(kvc == 0), stop=(kvc == 15))
                nc.tensor.matmul(oacc1[:], vNa[:, kvc * A + 128: kvc * A + 256],
                                 e1[:], start=(kvc == 0), stop=(kvc == 15))
            rinv = pstat.tile([1, L], f32, tag="rinv")
            nc.vector.reciprocal(rinv[:], sum_ps[:])
            brs = ppbc.tile([128, L], f32, tag="bcast")
            nc.tensor.matmul(brs[:], ones_row[:], rinv[:], start=True,
                             stop=True)
            brv = pscr.tile([128, L], f32, tag="brv")
            nc.vector.tensor_copy(brv[:], brs[:])
            # gate & residual (qTa becomes oT storage)
            nc.vector.tensor_mul(qTa[:, 0:L], oacc0[:], brv[:])
            nc.vector.tensor_mul(qTa[:, L:2 * L], oacc1[:], brv[:])
            for mc in range(2):
                ps = ppmm.tile([128, L], f32, tag="mm")
                for cc in range(2):
                    nc.tensor.matmul(
                        ps[:], agw_t[:, cc * A + mc * 128: cc * A + mc * 128 + 128],
                        hnloc[:, cc * L:(cc + 1) * L],
                        start=(cc == 0), stop=(cc == 1))
                sg = pscr.tile([128, L], bf16, tag="sg")
                nc.scalar.activation(sg[:], ps[:], AF.Sigmoid,
                                     bias=agb_t[:, mc:mc + 1])
                t = pscr.tile([128, L], f32, tag="gt")
                nc.vector.tensor_mul(t[:], sg[:], qTa[:, mc * L:(mc + 1) * L])
                nc.vector.tensor_add(xloc[:, mc * L:(mc + 1) * L],
                                     xloc[:, mc * L:(mc + 1) * L], t[:])

            # ln2 local -> hnloc
            ln_partition(xloc, L, 0, L, 2, alnw[:, 4:6], alnw[:, 6:8],
                         hnloc, L, 0, 1.0 / A)

            aff1_t = paw.tile([128, 2 * 1024], bf16, tag="aff1")
            for c in range(2):
                nc.sync.dma_start(aff1_t[:, c * 1024:(c + 1) * 1024],
                                  a_ff1[lyr, c])
            aff1b_t = paw.tile([128, 8], f32, tag="aff1b")
            nc.sync.dma_start(aff1b_t[:], a_ff1b[lyr])

            # ff1 + swiglu (a: chunks 0-3, g: 4-7)
            for j in range(4):
                a_ps = ppmm.tile([128, L], f32, tag="mm")
                for cc in range(2):
                    nc.tensor.matmul(
                        a_ps[:], aff1_t[:, cc * 1024 + j * 128: cc * 1024 + j * 128 + 128],
                        hnloc[:, cc * L:(cc + 1) * L],
                        start=(cc == 0), stop=(cc == 1))
                g_ps = ppmm.tile([128, L], f32, tag="mm")
                for cc in range(2):
                    nc.tensor.matmul(
                        g_ps[:], aff1_t[:, cc * 1024 + 512 + j * 128: cc * 1024 + 512 + j * 128 + 128],
                        hnloc[:, cc * L:(cc + 1) * L],
                        start=(cc == 0), stop=(cc == 1))
                sg = pscr.tile([128, L], bf16, tag="silu")
                nc.scalar.activation(sg[:], g_ps[:], AF.Silu,
                                     bias=aff1b_t[:, 4 + j:5 + j])
                av = pscr.tile([128, L], bf16, tag="av")
                nc.vector.tensor_scalar_add(av[:], a_ps[:],
                                            aff1b_t[:, j:j + 1])
                nc.vector.tensor_mul(swiA[:, j * L:(j + 1) * L], av[:], sg[:])

            aff2_t = paw.tile([128, 4 * A], bf16, tag="aff2")
            for c in range(4):
                nc.sync.dma_start(aff2_t[:, c * A:(c + 1) * A], a_ff2[lyr, c])
            aff2b_t = paw.tile([128, 2], f32, tag="aff2b")
            nc.sync.dma_start(aff2b_t[:], a_ff2b[lyr])

            for mc in range(2):
                ps = ppmm.tile([128, L], f32, tag="mm")
                for cc in range(4):
                    nc.tensor.matmul(
                        ps[:], aff2_t[:, cc * A + mc * 128: cc * A + mc * 128 + 128],
                        swiA[:, cc * L:(cc + 1) * L],
                        start=(cc == 0), stop=(cc == 3))
                t = pscr.tile([128, L], f32, tag="gt")
                nc.vector.tensor_scalar_add(t[:], ps[:],
                                            aff2b_t[:, mc:mc + 1])
                nc.vector.tensor_add(xloc[:, mc * L:(mc + 1) * L],
                                     xloc[:, mc * L:(mc + 1) * L], t[:])

            if lyr < ATOM_DEPTH - 1:
                atom_allgather()

        # final head: out_ln -> den^T
        oln_t = pconst.tile([128, 4], f32)
        nc.sync.dma_start(oln_t[:, 0:2], outln_d[0])
        nc.sync.dma_start(oln_t[:, 2:4], outln_d[1])
        outw_t = pconst.tile([128, 2 * 4], bf16)
        for c in range(2):
            nc.sync.dma_start(outw_t[:, c * 4:(c + 1) * 4], outw_d[c])
        ln_partition(xloc, L, 0, L, 2, oln_t[:, 0:2], oln_t[:, 2:4],
                     hnloc, L, 0, 1.0 / A)
        ps = ppmm.tile([128, L], f32, tag="mm")
        for cc in range(2):
            nc.tensor.matmul(ps[:4, :], outw_t[:, cc * 4:(cc + 1) * 4],
                             hnloc[:, cc * L:(cc + 1) * L],
                             start=(cc == 0), stop=(cc == 1))
        dsb = pscr.tile([4, L], f32, tag="den")
        nc.vector.tensor_copy(dsb[:], ps[:4, :])
        nc.sync.dma_start(denT_d[:], dsb[:])

        es.close()
    nc.compile()
    return nc


def _get_built():
    global _BUILT
    if _BUILT is None:
        _BUILT = _build()
    return _BUILT


# ---------------------------------------------------------------- host side
def _prep_core_inputs(x, aacodes, atcodes, aaindices, noised, nlev, params):
    """Returns list of 8 in_maps."""
    f32 = np.float32
    P = lambda t: np.asarray(t, dtype=f32)
    aacodes = np.asarray(aacodes).astype(np.int64)
    atcodes = np.asarray(atcodes).astype(np.int64)
    aaindices = np.asarray(aaindices).astype(np.int64)
    x = P(x); noised = P(noised); nlev = P(nlev)

    bias = _alibi_bias()                      # [H,L,L]
    s_seq = np.float32(DK ** -0.25)
    s_atom = np.float32((A // 8) ** -0.25)

    pos = _pos_encoding(aaindices, A)
    nl = {k: P(v) for k, v in params['nlev'].items()}
    four = _fourier_embed(nlev, nl)
    coordscale = np.sqrt(nlev[:, None, None] ** 2 + VARDATA)
    embed = (P(params['aa_embed'])[aacodes][aaindices][None]
             + P(params['atom_embed'])[atcodes][None]
             + pos[None] + four[:, None]
             + (noised / coordscale) @ P(params['coord_embed']))  # [B,NA,A]
    G = np.zeros((L, NA), f32)
    G[aaindices, np.arange(NA)] = 1.0

    x0 = _ln_np(x, P(params['norm1_g']), P(params['norm1_b']))    # [B,L,D]

    def bf(a):
        return np.ascontiguousarray(a).astype(BF16)

    def pcols(v, nch):      # feature vec [nch*128] -> [128, nch] f32
        return np.ascontiguousarray(np.asarray(v, f32).reshape(nch, 128).T)

    # ---- shared (per-layer) weights ----
    seq_w = dict(s_wqkv=[], s_wo=[], s_gw=[], s_gb=[], s_ln=[], s_ff1=[],
                 s_ff2=[])
    for p in params['seq']:
        seq_w['s_gb'].append(pcols(p['gb'], 8))
        seq_w['s_ln'].append(np.stack([pcols(p['ln1_g'], 8),
                                       pcols(p['ln1_b'], 8),
                                       pcols(p['ln2_g'], 8),
                                       pcols(p['ln2_b'], 8)]))
        seq_w['s_gw'].append(bf(P(p['gw']).reshape(8, 128, D)))
    atom_w = dict(a_wqkv=[], a_gw=[], a_gb=[], a_ln=[], a_ff1=[], a_ff1b=[],
                  a_ff2=[], a_ff2b=[])
    for p in params['atom']:
        w768 = np.concatenate([P(p['wq']) * s_atom, P(p['wk']) * s_atom,
                               P(p['wv'])], 1)
        atom_w['a_wqkv'].append(bf(w768.reshape(2, 128, 768)))
        atom_w['a_gw'].append(bf(P(p['gw']).reshape(2, 128, A)))
        atom_w['a_gb'].append(pcols(p['gb'], 2))
        atom_w['a_ln'].append(np.stack([pcols(p['ln1_g'], 2),
                                        pcols(p['ln1_b'], 2),
                                        pcols(p['ln2_g'], 2),
                                        pcols(p['ln2_b'], 2)]))
        atom_w['a_ff1'].append(bf(P(p['ff1']).reshape(2, 128, 1024)))
        atom_w['a_ff1b'].append(pcols(p['ff1_b'], 8))
        atom_w['a_ff2'].append(bf(P(p['ff2']).reshape(4, 128, A)))
        atom_w['a_ff2b'].append(pcols(p['ff2_b'], 2))
    shared = {k: np.stack(v) for k, v in {**seq_w, **atom_w}.items()
              if k not in ('s_wqkv', 's_wo', 's_ff1', 's_ff2')}
    shared['w4'] = np.ascontiguousarray(
        np.concatenate([P(params['to_coords']), P(params['to_confs'])],
                       1).reshape(8, 128, 4))
    shared['to_atom'] = np.ascontiguousarray(
        P(params['to_atom']).reshape(8, 128, A))
    shared['norm2'] = np.stack([P(params['norm2_g'])[None],
                                P(params['norm2_b'])[None]])
    shared['norm3'] = np.stack([pcols(params['norm3_g'], 2),
                                pcols(params['norm3_b'], 2)])
    shared['out_lnw'] = np.stack([pcols(params['out_ln_g'], 2),
                                  pcols(params['out_ln_b'], 2)])
    ow = np.concatenate([P(params['out_w']), np.zeros((A, 1), f32)], 1)
    shared['out_w'] = bf(ow.reshape(2, 128, 4))

    in_maps = []
    for c in range(N_CORES):
        b, tp = c // 4, c % 4
        m = dict(shared)
        m['xT0'] = np.ascontiguousarray(x0[b].T.reshape(8, 128, L))
        wqkv_l, wo_l, ff1_l, ff2_l = [], [], [], []
        for p in params['seq']:
            sl = slice(tp * 256, (tp + 1) * 256)
            w768 = np.concatenate([P(p['wq'])[:, sl] * s_seq,
                                   P(p['wk'])[:, sl] * s_seq,
                                   P(p['wv'])[:, sl]], 1)
            wqkv_l.append(bf(w768.reshape(8, 128, 768)))
            wo_l.append(bf(P(p['wo'])[sl].reshape(2, 128, D)))
            f1 = P(p['ff1'])
            a_w = f1[:, tp * 1024:(tp + 1) * 1024]
            g_w = f1[:, 4096 + tp * 1024:4096 + (tp + 1) * 1024]
            ff1_l.append(bf(np.concatenate([a_w, g_w], 1).reshape(8, 128, 2048)))
            ff2_l.append(bf(P(p['ff2'])[tp * 1024:(tp + 1) * 1024]
                            .reshape(8, 128, D)))
        m['s_wqkv'] = np.stack(wqkv_l)
        m['s_wo'] = np.stack(wo_l)
        m['s_ff1'] = np.stack(ff1_l)
        m['s_ff2'] = np.stack(ff2_l)
        eb = np.exp(bias[tp * 4:(tp + 1) * 4])          # [4,L,L] (q,kv)
        m['expb'] = bf(np.ascontiguousarray(
            eb.transpose(0, 2, 1).reshape(4, 4, 128, L)))
        m['g_loc'] = bf(np.ascontiguousarray(
            G[:, tp * L:(tp + 1) * L].reshape(4, 128, L)))
        m['embedT'] = np.ascontiguousarray(
            embed[b, tp * L:(tp + 1) * L].T.reshape(2, 128, L))
        in_maps.append(m)
    return in_maps, G


def kernel(x, aacodes, atcodes, aaindices, noised_coords_in, nlev_in, params):
    from concourse.bass_utils import run_bass_kernel_spmd
    nc = _get_built()
    in_maps, _ = _prep_core_inputs(x, aacodes, atcodes, aaindices,
                                   noised_coords_in, nlev_in, params)
    res = run_bass_kernel_spmd(nc, in_maps, core_ids=list(range(N_CORES)))
    outs = res.results

    noised = np.asarray(noised_coords_in, np.float32)
    nlev = np.asarray(nlev_in, np.float32)
    pred_coords = np.zeros((B, L, 3), np.float32)
    pred_confs = np.zeros((B, L), np.float32)
    den = np.zeros((B, NA, 3), np.float32)
    for b in range(B):
        c4 = outs[b * 4]['coords4'].reshape(L, 4)
        pred_coords[b] = c4[:, :3]
        pred_confs[b] = c4[:, 3]
        for tp in range(4):
            dT = outs[b * 4 + tp]['denT']        # [4, L]
            den[b, tp * L:(tp + 1) * L] = dT[:3].T
    t_h = nlev[:, None, None]
    pred_denoised = (den * SIGDATA * t_h / np.sqrt(VARDATA + t_h ** 2)
                     + noised * VARDATA / (VARDATA + t_h ** 2))
    return pred_denoised, pred_coords, pred_confs


# revision 20
# speedup vs baseline: 1.0776x; 1.0776x over previous
"""Trainium2 Bass kernel for nn_DiffusionNet (8 NeuronCores).

Sharding: 2 batch groups x 4 cores.
  Seq stack (24 layer-apps): TP over heads/hidden, bf16 AllReduce after wo/ff2.
  Atom stack (10 layers): token-parallel (512/core), replicated K/V, AllGather.
Activations kept feature-major ("transposed" [feat, tok]); every GEMM is
lhsT=weight-chunk [128,128] x rhs=activation [128,512]. Matmul I/O bf16,
PSUM/residual f32.
"""
import sys
sys.path.insert(0, '/opt/trn_rl_repo')
import numpy as np
import ml_dtypes

BF16 = ml_dtypes.bfloat16

D, A, H, DK = 1024, 256, 16, 64
L, NA, B = 512, 2048, 2
SEQ_DEPTH, CYCLES, ATOM_DEPTH = 8, 3, 10
EPS = 1e-5
SIGDATA, VARDATA = 16.0, 256.0
N_CORES = 8
GROUPS = [[0, 1, 2, 3], [4, 5, 6, 7]]

_BUILT = None


# ---------------------------------------------------------------- host math
def _ln_np(x, g, b):
    m = x.mean(-1, keepdims=True)
    v = x.var(-1, keepdims=True)
    return (x - m) / np.sqrt(v + EPS) * g + b


def _alibi_bias():
    def pow2(n):
        start = 2.0 ** (-2.0 ** (-(np.log2(n) - 3)))
        return [start * start ** i for i in range(n)]
    if np.log2(H).is_integer():
        s = pow2(H)
    else:
        c = 2 ** int(np.floor(np.log2(H)))
        s = pow2(c) + pow2(2 * c)[0::2][: H - c]
    slopes = np.asarray(s, dtype=np.float32)
    idx = np.arange(L)
    rel = np.abs(idx[None, :] - idx[:, None]).astype(np.float32)
    return -slopes[:, None, None] * rel[None]  # [H,L,L]


def _pos_encoding(positions, d):
    pos = positions.astype(np.float32)[:, None]
    div = np.exp(np.arange(0, d, 2, dtype=np.float32) * (-np.log(10000.0) / d))
    ang = pos * div
    return np.stack([np.sin(ang), np.cos(ang)], -1).reshape(positions.shape[0], d)


def _fourier_embed(nlev, p):
    scales = 2.0 ** np.arange(8, dtype=np.float32)
    xs = nlev[:, None] / scales
    f = np.concatenate([np.sin(xs), np.cos(xs)], axis=-1).astype(np.float32)
    return _ln_np(f @ p['w'] + p['b'], p['g'], p['beta'])


# ---------------------------------------------------------------- builder
def _build():
    import concourse.bacc as bacc
    import concourse.mybir as mybir
    import concourse.tile as tile
    from contextlib import ExitStack

    dt = mybir.dt
    AF = mybir.ActivationFunctionType
    OP = mybir.AluOpType
    f32, bf16 = dt.float32, dt.bfloat16
    nc = bacc.Bacc("TRN2", target_bir_lowering=False, debug=False,
                   num_devices=N_CORES)

    def din(name, shape, d=bf16):
        return nc.dram_tensor(name, list(shape), d, kind="ExternalInput")

    xT0 = din("xT0", [8, 128, L], f32)
    s_wqkv = din("s_wqkv", [SEQ_DEPTH, 8, 128, 768])
    s_wo = din("s_wo", [SEQ_DEPTH, 2, 128, D])
    s_gw = din("s_gw", [SEQ_DEPTH, 8, 128, D])
    s_gb = din("s_gb", [SEQ_DEPTH, 128, 8], f32)
    s_ln = din("s_ln", [SEQ_DEPTH, 4, 128, 8], f32)
    s_ff1 = din("s_ff1", [SEQ_DEPTH, 8, 128, 2048])
    s_ff2 = din("s_ff2", [SEQ_DEPTH, 8, 128, D])
    expb_d = din("expb", [4, 4, 128, L])
    w4_d = din("w4", [8, 128, 4], f32)
    toat_d = din("to_atom", [8, 128, A], f32)
    norm2_d = din("norm2", [2, 1, A], f32)
    norm3_d = din("norm3", [2, 128, 2], f32)
    gloc_d = din("g_loc", [4, 128, L])
    embT_d = din("embedT", [2, 128, L], f32)
    a_wqkv = din("a_wqkv", [ATOM_DEPTH, 2, 128, 768])
    a_gw = din("a_gw", [ATOM_DEPTH, 2, 128, A])
    a_gb = din("a_gb", [ATOM_DEPTH, 128, 2], f32)
    a_lnw = din("a_ln", [ATOM_DEPTH, 4, 128, 2], f32)
    a_ff1 = din("a_ff1", [ATOM_DEPTH, 2, 128, 1024])
    a_ff1b = din("a_ff1b", [ATOM_DEPTH, 128, 8], f32)
    a_ff2 = din("a_ff2", [ATOM_DEPTH, 4, 128, A])
    a_ff2b = din("a_ff2b", [ATOM_DEPTH, 128, 2], f32)
    outln_d = din("out_lnw", [2, 128, 2], f32)
    outw_d = din("out_w", [2, 128, 4])

    coords4 = nc.dram_tensor("coords4", [4, 128, 4], f32, kind="ExternalOutput")
    denT_d = nc.dram_tensor("denT", [4, L], f32, kind="ExternalOutput")

    with tile.TileContext(nc) as tc, \
            nc.allow_low_precision(reason="bf16 compute precision by design"):
        es = ExitStack()
        pconst = es.enter_context(tc.tile_pool(name="pconst", bufs=1))
        pact = es.enter_context(tc.tile_pool(name="pact", bufs=1))
        pscr = es.enter_context(tc.tile_pool(name="pscr", bufs=2))
        pstat = es.enter_context(tc.tile_pool(name="pstat", bufs=2))
        ppmm = es.enter_context(tc.tile_pool(name="ppmm", bufs=2, space="PSUM"))
        pprow = es.enter_context(tc.tile_pool(name="pprow", bufs=2, space="PSUM"))
        ppbc = es.enter_context(tc.tile_pool(name="ppbc", bufs=2, space="PSUM"))
        ppo = es.enter_context(tc.tile_pool(name="ppo", bufs=2, space="PSUM"))
        pdram = es.enter_context(tc.tile_pool(name="pdram", bufs=1, space="DRAM"))

        ones_col = pconst.tile([128, 1], bf16)
        nc.vector.memset(ones_col[:], 1.0)
        ones_row = pconst.tile([1, 128], f32)
        nc.vector.memset(ones_row[:], 1.0)
        ones_row_b = pconst.tile([1, 128], bf16)
        nc.vector.memset(ones_row_b[:], 1.0)
        eps_col = pconst.tile([128, 1], f32)
        nc.vector.memset(eps_col[:], EPS)

        xT = pact.tile([128, 8 * L], f32, name="xT")
        hnT = pact.tile([128, 8 * L], bf16, name="hnT")
        hnloc = pact.tile([128, 2 * L], bf16, name="hnloc")
        a_nat = pact.tile([128, 4 * A], bf16, name="a_nat")
        emb_t = pact.tile([128, 2 * L], f32, name="emb_t")
        xloc = pact.tile([128, 2 * L], f32, name="xloc")
        obuf = pact.tile([128, 8 * L], bf16, name="obuf")

        bnc_in = [pdram.tile([128, 4 * L], bf16, name=f"bnc_in{i}")
                  for i in range(2)]
        bnc_out = [pdram.tile([128, 4 * L], bf16, name=f"bnc_out{i}")
                   for i in range(2)]
        ag_in = pdram.tile([128, 2 * L], f32, name="ag_in")
        ag_out = pdram.tile([4, 128, 2 * L], f32, name="ag_out")

        for c in range(8):
            nc.sync.dma_start(xT[:, c * L:(c + 1) * L], xT0[c])

        # ---------------------------------------------- helpers
        def ln_partition(src_tile, src_stride, tok_off, T, nchunks, g_col,
                         b_col, dst_tile, dst_stride, dst_off, inv_dim):
            """LN over partition axis (features = nchunks*128) for T tokens.
            src chunk c: src_tile[:, c*src_stride + tok_off : +T] (f32).
            dst chunk c: dst_tile[:, c*dst_stride + dst_off : +T] (bf16)."""
            s_ps = pprow.tile([1, T], f32, tag="lnrow")
            ss_ps = pprow.tile([1, T], f32, tag="lnrow")
            for c in range(nchunks):
                sl = src_tile[:, c * src_stride + tok_off:
                              c * src_stride + tok_off + T]
                xb = pscr.tile([128, T], bf16, tag="lnxb")
                nc.vector.tensor_copy(xb[:], sl)
                sq = pscr.tile([128, T], bf16, tag="lnsq")
                nc.vector.tensor_mul(sq[:], xb[:], xb[:])
                nc.tensor.matmul(s_ps[:], ones_col[:], xb[:],
                                 start=(c == 0), stop=(c == nchunks - 1))
                nc.tensor.matmul(ss_ps[:], ones_col[:], sq[:],
                                 start=(c == 0), stop=(c == nchunks - 1))
            mean = pstat.tile([1, T], bf16, tag="mean")
            nc.vector.tensor_scalar_mul(mean[:], s_ps[:], inv_dim)
            tmp = pstat.tile([1, T], f32, tag="stmp")
            nc.vector.tensor_scalar_mul(tmp[:], ss_ps[:], inv_dim)
            m2 = pstat.tile([1, T], f32, tag="sm2")
            nc.vector.tensor_mul(m2[:], mean[:], mean[:])
            nc.vector.tensor_sub(tmp[:], tmp[:], m2[:])
            nc.scalar.activation(tmp[:], tmp[:], AF.Sqrt,
                                 bias=eps_col[:1, 0:1])
            rstd = pstat.tile([1, T], bf16, tag="rstd")
            nc.vector.reciprocal(rstd[:], tmp[:])
            bm = ppbc.tile([128, T], f32, tag="bcast")
            nc.tensor.matmul(bm[:], ones_row_b[:], mean[:], start=True,
                             stop=True)
            br = ppbc.tile([128, T], f32, tag="bcast")
            nc.tensor.matmul(br[:], ones_row_b[:], rstd[:], start=True,
                             stop=True)
            # fused wide (x - mean) over all chunks, bf16 out
            uw = pscr.tile([128, nchunks * T], bf16, tag="lnu", bufs=1)
            if src_stride == T and tok_off == 0:
                src3 = src_tile[:, 0:nchunks * T].rearrange(
                    "p (c t) -> p c t", c=nchunks)
            else:
                src3 = src_tile[:, 0:nchunks * src_stride].rearrange(
                    "p (c x) -> p c x", c=nchunks)[:, :, tok_off:tok_off + T]
            uw3 = uw[:].rearrange("p (c t) -> p c t", c=nchunks)
            bm3 = bm[:].rearrange("p (o t) -> p o t", o=1).to_broadcast(
                (128, nchunks, T))
            nc.vector.tensor_sub(uw3, src3, bm3)
            for c in range(nchunks):
                dsl = dst_tile[:, c * dst_stride + dst_off:
                               c * dst_stride + dst_off + T]
                nc.vector.scalar_tensor_tensor(
                    dsl, uw[:, c * T:(c + 1) * T], g_col[:, c:c + 1], br[:],
                    op0=OP.mult, op1=OP.mult)
                nc.vector.tensor_scalar_add(dsl, dsl, b_col[:, c:c + 1])

        # ================================================== SEQ PHASE
        with ExitStack() as seq_es:
            pseq = seq_es.enter_context(tc.tile_pool(name="pseq", bufs=1))
            pw_qkv = seq_es.enter_context(tc.tile_pool(name="pw_qkv", bufs=1))
            pw_wo = seq_es.enter_context(tc.tile_pool(name="pw_wo", bufs=1))
            pw_gw = seq_es.enter_context(tc.tile_pool(name="pw_gw", bufs=1))
            pw_ffq = seq_es.enter_context(tc.tile_pool(name="pw_ffq", bufs=2))
            pw_ln = seq_es.enter_context(tc.tile_pool(name="pw_ln", bufs=2))

            qT = pseq.tile([128, 2 * L], bf16, name="qT")
            kT = pseq.tile([128, 2 * L], bf16, name="kT")
            vN = pseq.tile([128, 4 * A], bf16, name="vN")
            oT = pseq.tile([128, 2 * L], bf16, name="oT")
            swiT = pseq.tile([128, 8 * L], bf16, name="swiT")
            expb_sb = pseq.tile([128, 16 * L], bf16, name="expb_sb")
            for h in range(4):
                for kvc in range(4):
                    i = h * 4 + kvc
                    nc.sync.dma_start(expb_sb[:, i * L:(i + 1) * L],
                                      expb_d[h, kvc])

            for visit in range(CYCLES * SEQ_DEPTH):
                lyr = visit % SEQ_DEPTH
                wqkv = pw_qkv.tile([128, 8 * 768], bf16, tag="qkv")
                for c in range(8):
                    nc.sync.dma_start(wqkv[:, c * 768:(c + 1) * 768],
                                      s_wqkv[lyr, c])
                lnw = pw_ln.tile([128, 4 * 8], f32, tag="ln")
                for i in range(4):
                    nc.sync.dma_start(lnw[:, i * 8:(i + 1) * 8], s_ln[lyr, i])
                gbc = pw_ln.tile([128, 8], f32, tag="gb")
                nc.sync.dma_start(gbc[:], s_gb[lyr])

                # ln1 -> hnT
                ln_partition(xT, L, 0, L, 8, lnw[:, 0:8], lnw[:, 8:16],
                             hnT, L, 0, 1.0 / D)

                # q^T/k^T head-pair packs; v natural
                for pair in range(2):
                    for which, dstt in ((0, qT), (2, kT)):
                        ps = ppmm.tile([128, L], f32, tag="mm")
                        for cc in range(8):
                            co = cc * 768 + (which + pair) * 128
                            nc.tensor.matmul(ps[:], wqkv[:, co:co + 128],
                                             hnT[:, cc * L:(cc + 1) * L],
                                             start=(cc == 0), stop=(cc == 7))
                        nc.vector.tensor_copy(
                            dstt[:, pair * L:(pair + 1) * L], ps[:])
                for tci in range(4):
                    ps = ppmm.tile([128, A], f32, tag="mm")
                    for cc in range(8):
                        nc.tensor.matmul(
                            ps[:],
                            hnT[:, cc * L + tci * 128: cc * L + tci * 128 + 128],
                            wqkv[:, cc * 768 + 512: cc * 768 + 768],
                            start=(cc == 0), stop=(cc == 7))
                    nc.vector.tensor_copy(vN[:, tci * A:(tci + 1) * A], ps[:])

                wo_t = pw_wo.tile([128, 2 * D], bf16, tag="wo")
                for c in range(2):
                    nc.sync.dma_start(wo_t[:, c * D:(c + 1) * D], s_wo[lyr, c])

                # attention per local head
                for h in range(4):
                    po = (h % 2) * 64
                    cb = (h // 2) * L
                    sum_ps = pprow.tile([1, L], f32, tag="lnrow")
                    oacc = ppo.tile([64, L], f32, tag="oacc")
                    for kvc in range(4):
                        ps = ppmm.tile([128, L], f32, tag="mm")
                        nc.tensor.matmul(
                            ps[:],
                            kT[po:po + 64, cb + kvc * 128: cb + kvc * 128 + 128],
                            qT[po:po + 64, cb: cb + L], start=True, stop=True)
                        e1 = pscr.tile([128, L], bf16, tag="es1")
                        nc.scalar.activation(e1[:], ps[:], AF.Exp)
                        e2 = pscr.tile([128, L], bf16, tag="es2")
                        i = h * 4 + kvc
                        nc.vector.tensor_mul(e2[:], e1[:],
                                             expb_sb[:, i * L:(i + 1) * L])
                        nc.tensor.matmul(sum_ps[:], ones_col[:], e2[:],
                                         start=(kvc == 0), stop=(kvc == 3))
                        nc.tensor.matmul(
                            oacc[:], vN[:, kvc * A + h * 64: kvc * A + h * 64 + 64],
                            e2[:], start=(kvc == 0), stop=(kvc == 3))
                    rinv = pstat.tile([1, L], bf16, tag="rinv")
                    nc.vector.reciprocal(rinv[:], sum_ps[:])
                    brs = ppbc.tile([64, L], f32, tag="bcast")
                    nc.tensor.matmul(brs[:], ones_row_b[:, :64], rinv[:],
                                     start=True, stop=True)
                    brv = pscr.tile([64, L], f32, tag="brv")
                    nc.vector.tensor_copy(brv[:], brs[:])
                    nc.vector.tensor_mul(oT[po:po + 64, cb:cb + L], oacc[:],
                                         brv[:])

                # wo partial -> obuf (bf16), AllReduce
                for mc in range(8):
                    ps = ppmm.tile([128, L], f32, tag="mm")
                    for cc in range(2):
                        nc.tensor.matmul(
                            ps[:], wo_t[:, cc * D + mc * 128: cc * D + mc * 128 + 128],
                            oT[:, cc * L:(cc + 1) * L],
                            start=(cc == 0), stop=(cc == 1))
                    nc.vector.tensor_copy(obuf[:, mc * L:(mc + 1) * L], ps[:])

                gw_t = pw_gw.tile([128, 8 * D], bf16, tag="gw")
                for c in range(8):
                    nc.sync.dma_start(gw_t[:, c * D:(c + 1) * D], s_gw[lyr, c])

                for half in range(2):
                    hs = slice(half * 4 * L, (half + 1) * 4 * L)
                    nc.sync.dma_start(bnc_in[half][:], obuf[:, hs])
                    nc.gpsimd.collective_compute(
                        "AllReduce", OP.add, replica_groups=GROUPS,
                        ins=[bnc_in[half][:]], outs=[bnc_out[half][:]])
                    nc.sync.dma_start(obuf[:, hs], bnc_out[half][:])

                # gate & residual (paired chunks for wide DVE ops)
                for mp in range(4):
                    sg = pscr.tile([128, 2 * L], bf16, tag="sg")
                    for k in range(2):
                        mc = mp * 2 + k
                        ps = ppmm.tile([128, L], f32, tag="mm")
                        for cc in range(8):
                            nc.tensor.matmul(
                                ps[:], gw_t[:, cc * D + mc * 128: cc * D + mc * 128 + 128],
                                hnT[:, cc * L:(cc + 1) * L],
                                start=(cc == 0), stop=(cc == 7))
                        nc.scalar.activation(sg[:, k * L:(k + 1) * L], ps[:],
                                             AF.Sigmoid,
                                             bias=gbc[:, mc:mc + 1])
                    t = pscr.tile([128, 2 * L], f32, tag="gt")
                    wsl = slice(mp * 2 * L, (mp + 1) * 2 * L)
                    nc.vector.tensor_mul(t[:], sg[:], obuf[:, wsl])
                    nc.vector.tensor_add(xT[:, wsl], xT[:, wsl], t[:])

                # ln2 -> hnT
                ln_partition(xT, L, 0, L, 8, lnw[:, 16:24], lnw[:, 24:32],
                             hnT, L, 0, 1.0 / D)

                # ff1 in quarters (a: q 0-3 -> swiT; g: silu * into swiT)
                for half, base in ((0, 0), (1, 1024)):
                    for q in range(2):
                        ffq = pw_ffq.tile([128, 8 * 512], bf16, tag="ffq")
                        for cc in range(8):
                            src = s_ff1[lyr, cc][:, base + q * 512:
                                                 base + (q + 1) * 512]
                            nc.sync.dma_start(
                                ffq[:, cc * 512:(cc + 1) * 512], src)
                        for j in range(4):
                            mc = q * 4 + j
                            ps = ppmm.tile([128, L], f32, tag="mm")
                            for cc in range(8):
                                nc.tensor.matmul(
                                    ps[:],
                                    ffq[:, cc * 512 + j * 128: cc * 512 + j * 128 + 128],
                                    hnT[:, cc * L:(cc + 1) * L],
                                    start=(cc == 0), stop=(cc == 7))
                            if half == 0:
                                nc.vector.tensor_copy(
                                    swiT[:, mc * L:(mc + 1) * L], ps[:])
                            else:
                                sg = pscr.tile([128, L], bf16, tag="silu")
                                nc.scalar.activation(sg[:], ps[:], AF.Silu)
                                nc.vector.tensor_mul(
                                    swiT[:, mc * L:(mc + 1) * L],
                                    swiT[:, mc * L:(mc + 1) * L], sg[:])

                for hh in range(2):
                    ff2h = pw_ffq.tile([128, 8 * 512], bf16, tag="ffq")
                    for cc in range(8):
                        nc.sync.dma_start(
                            ff2h[:, cc * 512:(cc + 1) * 512],
                            s_ff2[lyr, cc][:, hh * 512:(hh + 1) * 512])
                    for j in range(4):
                        mc = hh * 4 + j
                        ps = ppmm.tile([128, L], f32, tag="mm")
                        for cc in range(8):
                            nc.tensor.matmul(
                                ps[:],
                                ff2h[:, cc * 512 + j * 128: cc * 512 + j * 128 + 128],
                                swiT[:, cc * L:(cc + 1) * L],
                                start=(cc == 0), stop=(cc == 7))
                        nc.vector.tensor_copy(obuf[:, mc * L:(mc + 1) * L],
                                              ps[:])
                    hs = slice(hh * 4 * L, (hh + 1) * 4 * L)
                    nc.sync.dma_start(bnc_in[hh][:], obuf[:, hs])
                    nc.gpsimd.collective_compute(
                        "AllReduce", OP.add, replica_groups=GROUPS,
                        ins=[bnc_in[hh][:]], outs=[bnc_out[hh][:]])
                    nc.sync.dma_start(obuf[:, hs], bnc_out[hh][:])
                    nc.vector.tensor_add(xT[:, hs], xT[:, hs], obuf[:, hs])

            # ---------------- heads + embed (still in seq scope) ----------
            w4_t = pconst.tile([128, 8 * 4], f32)
            for c in range(8):
                nc.sync.dma_start(w4_t[:, c * 4:(c + 1) * 4], w4_d[c])
            for tci in range(4):
                ps = ppmm.tile([128, 4], f32, tag="mm")
                for cc in range(8):
                    nc.tensor.matmul(
                        ps[:], xT[:, cc * L + tci * 128: cc * L + tci * 128 + 128],
                        w4_t[:, cc * 4:(cc + 1) * 4],
                        start=(cc == 0), stop=(cc == 7))
                ot = pscr.tile([128, 4], f32, tag="c4")
                nc.vector.tensor_copy(ot[:], ps[:])
                nc.sync.dma_start(coords4[tci], ot[:])

            toat_t = pconst.tile([128, 8 * A], f32)
            for c in range(8):
                nc.sync.dma_start(toat_t[:, c * A:(c + 1) * A], toat_d[c])
            n2g = pconst.tile([1, A], f32)
            nc.sync.dma_start(n2g[:], norm2_d[0])
            n2b = pconst.tile([1, A], f32)
            nc.sync.dma_start(n2b[:], norm2_d[1])
            n2g_sb = pconst.tile([128, A], f32)
            n2b_sb = pconst.tile([128, A], f32)
            bc = ppbc.tile([128, A], f32, tag="bcast")
            nc.tensor.matmul(bc[:], ones_row[:], n2g[:], start=True, stop=True)
            nc.vector.tensor_copy(n2g_sb[:], bc[:])
            bc2 = ppbc.tile([128, A], f32, tag="bcast")
            nc.tensor.matmul(bc2[:], ones_row[:], n2b[:], start=True, stop=True)
            nc.vector.tensor_copy(n2b_sb[:], bc2[:])

            for tci in range(4):
                ps = ppmm.tile([128, A], f32, tag="mm")
                for cc in range(8):
                    nc.tensor.matmul(
                        ps[:], xT[:, cc * L + tci * 128: cc * L + tci * 128 + 128],
                        toat_t[:, cc * A:(cc + 1) * A],
                        start=(cc == 0), stop=(cc == 7))
                s_r = pstat.tile([128, 1], f32, tag="nr")
                nc.vector.reduce_sum(s_r[:], ps[:], axis=mybir.AxisListType.X)
                mean = pstat.tile([128, 1], f32, tag="nm")
                nc.vector.tensor_scalar_mul(mean[:], s_r[:], 1.0 / A)
                sq = pscr.tile([128, A], f32, tag="nsq")
                nc.scalar.activation(sq[:], ps[:], AF.Square)
                ss_r = pstat.tile([128, 1], f32, tag="nss")
                nc.vector.reduce_sum(ss_r[:], sq[:], axis=mybir.AxisListType.X)
                ex2 = pstat.tile([128, 1], f32, tag="ne2")
                nc.vector.tensor_scalar_mul(ex2[:], ss_r[:], 1.0 / A)
                m2 = pstat.tile([128, 1], f32, tag="nm2")
                nc.vector.tensor_mul(m2[:], mean[:], mean[:])
                nc.vector.tensor_sub(ex2[:], ex2[:], m2[:])
                nc.scalar.activation(ex2[:], ex2[:], AF.Sqrt,
                                     bias=eps_col[:])
                rstd = pstat.tile([128, 1], f32, tag="nrs")
                nc.vector.reciprocal(rstd[:], ex2[:])
                t1 = pscr.tile([128, A], f32, tag="nt1")
                nc.vector.tensor_scalar(t1[:], ps[:], mean[:], rstd[:],
                                        op0=OP.subtract, op1=OP.mult)
                t2 = pscr.tile([128, A], f32, tag="nt2")
                nc.vector.tensor_mul(t2[:], t1[:], n2g_sb[:])
                nc.vector.tensor_add(a_nat[:, tci * A:(tci + 1) * A], t2[:],
                                     n2b_sb[:])

            gl_t = pconst.tile([128, 4 * L], bf16)
            for c in range(4):
                nc.sync.dma_start(gl_t[:, c * L:(c + 1) * L], gloc_d[c])
            for c in range(2):
                nc.sync.dma_start(emb_t[:, c * L:(c + 1) * L], embT_d[c])
            n3 = pconst.tile([128, 4], f32)
            nc.sync.dma_start(n3[:, 0:2], norm3_d[0])
            nc.sync.dma_start(n3[:, 2:4], norm3_d[1])

            for mc in range(2):
                ps = ppmm.tile([128, L], f32, tag="mm")
                for cc in range(4):
                    nc.tensor.matmul(
                        ps[:], a_nat[:, cc * A + mc * 128: cc * A + mc * 128 + 128],
                        gl_t[:, cc * L:(cc + 1) * L],
                        start=(cc == 0), stop=(cc == 3))
                nc.vector.tensor_add(xloc[:, mc * L:(mc + 1) * L], ps[:],
                                     emb_t[:, mc * L:(mc + 1) * L])
            ln_partition(xloc, L, 0, L, 2, n3[:, 0:2], n3[:, 2:4],
                         hnloc, L, 0, 1.0 / A)
            for mc in range(2):
                nc.vector.tensor_copy(xloc[:, mc * L:(mc + 1) * L],
                                      hnloc[:, mc * L:(mc + 1) * L])

        # ================================================== ATOM PHASE
        patom = es.enter_context(tc.tile_pool(name="patom", bufs=1))
        paw = es.enter_context(tc.tile_pool(name="paw", bufs=2))
        qTa = patom.tile([128, 2 * L], bf16, name="qTa")     # also reused as oTa
        kTa = patom.tile([128, 2 * 2048], bf16, name="kTa")
        vNa = patom.tile([128, 16 * A], bf16, name="vNa")
        swiA = patom.tile([128, 4 * L], bf16, name="swiA")

        def atom_allgather():
            nc.sync.dma_start(ag_in[:], xloc[:])
            nc.gpsimd.collective_compute(
                "AllGather", OP.bypass, replica_groups=GROUPS,
                ins=[ag_in[:]], outs=[ag_out[:]])
            nc.sync.dma_start(
                xT[:, 0:4096].rearrange("p (c r t) -> p c r t", c=2, r=4),
                ag_out[:].rearrange("r p (c t) -> p c r t", c=2))

        atom_allgather()

        for lyr in range(ATOM_DEPTH):
            awqkv = paw.tile([128, 2 * 768], bf16, tag="aqkv")
            for c in range(2):
                nc.sync.dma_start(awqkv[:, c * 768:(c + 1) * 768],
                                  a_wqkv[lyr, c])
            alnw = paw.tile([128, 4 * 2], f32, tag="aln")
            for i in range(4):
                nc.sync.dma_start(alnw[:, i * 2:(i + 1) * 2], a_lnw[lyr, i])
            agw_t = paw.tile([128, 2 * A], bf16, tag="agw")
            for c in range(2):
                nc.sync.dma_start(agw_t[:, c * A:(c + 1) * A], a_gw[lyr, c])
            agb_t = paw.tile([128, 2], f32, tag="agb")
            nc.sync.dma_start(agb_t[:], a_gb[lyr])

            # ln1 full (4 token blocks) -> hnT [128, c*2048 + t]
            for tb in range(4):
                ln_partition(xT, 2048, tb * L, L, 2, alnw[:, 0:2],
                             alnw[:, 2:4], hnT, 2048, tb * L, 1.0 / A)
            # ln1 local -> hnloc
            ln_partition(xloc, L, 0, L, 2, alnw[:, 0:2], alnw[:, 2:4],
                         hnloc, L, 0, 1.0 / A)

            # qT local [A,512]
            for mc in range(2):
                ps = ppmm.tile([128, L], f32, tag="mm")
                for cc in range(2):
                    nc.tensor.matmul(
                        ps[:], awqkv[:, cc * 768 + mc * 128: cc * 768 + mc * 128 + 128],
                        hnloc[:, cc * L:(cc + 1) * L],
                        start=(cc == 0), stop=(cc == 1))
                nc.vector.tensor_copy(qTa[:, mc * L:(mc + 1) * L], ps[:])
            # kT full [A,2048]
            for mc in range(2):
                for tb in range(4):
                    ps = ppmm.tile([128, L], f32, tag="mm")
                    for cc in range(2):
                        nc.tensor.matmul(
                            ps[:],
                            awqkv[:, cc * 768 + 256 + mc * 128: cc * 768 + 256 + mc * 128 + 128],
                            hnT[:, cc * 2048 + tb * L: cc * 2048 + (tb + 1) * L],
                            start=(cc == 0), stop=(cc == 1))
                    nc.vector.tensor_copy(
                        kTa[:, mc * 2048 + tb * L: mc * 2048 + (tb + 1) * L],
                        ps[:])
            # v natural full [2048, A]
            for tkc in range(16):
                ps = ppmm.tile([128, A], f32, tag="mm")
                for cc in range(2):
                    nc.tensor.matmul(
                        ps[:],
                        hnT[:, cc * 2048 + tkc * 128: cc * 2048 + tkc * 128 + 128],
                        awqkv[:, cc * 768 + 512: cc * 768 + 768],
                        start=(cc == 0), stop=(cc == 1))
                nc.vector.tensor_copy(vNa[:, tkc * A:(tkc + 1) * A], ps[:])

            # attention: scores^T chunks, exp, sum, o' accumulation
            sum_ps = pprow.tile([1, L], f32, tag="lnrow")
            oacc0 = ppo.tile([128, L], f32, tag="oacc")
            oacc1 = ppo.tile([128, L], f32, tag="oacc")
            for kvc in range(16):
                ps = ppmm.tile([128, L], f32, tag="mm")
                for cc in range(2):
                    nc.tensor.matmul(
                        ps[:], kTa[:, cc * 2048 + kvc * 128: cc * 2048 + kvc * 128 + 128],
                        qTa[:, cc * L:(cc + 1) * L],
                        start=(cc == 0), stop=(cc == 1))
                e1 = pscr.tile([128, L], bf16, tag="es1")
                nc.scalar.activation(e1[:], ps[:], AF.Exp)
                nc.tensor.matmul(sum_ps[:], ones_col[:], e1[:],
                                 start=(kvc == 0), stop=(kvc == 15))
                nc.tensor.matmul(oacc0[:], vNa[:, kvc * A: kvc * A + 128],
                                 e1[:], start=(kvc == 0), stop=(kvc == 15))
                nc.tensor.matmul(oacc1[:], vNa[:, kvc * A + 128: kvc * A + 256],
                                 e1[:], start=(kvc == 0), stop=(kvc == 15))
            rinv = pstat.tile([1, L], bf16, tag="rinv")
            nc.vector.reciprocal(rinv[:], sum_ps[:])
            brs = ppbc.tile([128, L], f32, tag="bcast")
            nc.tensor.matmul(brs[:], ones_row_b[:], rinv[:], start=True,
                             stop=True)
            brv = pscr.tile([128, L], f32, tag="brv")
            nc.vector.tensor_copy(brv[:], brs[:])
            # gate & residual (qTa becomes oT storage)
            nc.vector.tensor_mul(qTa[:, 0:L], oacc0[:], brv[:])
            nc.vector.tensor_mul(qTa[:, L:2 * L], oacc1[:], brv[:])
            for mc in range(2):
                ps = ppmm.tile([128, L], f32, tag="mm")
                for cc in range(2):
                    nc.tensor.matmul(
                        ps[:], agw_t[:, cc * A + mc * 128: cc * A + mc * 128 + 128],
                        hnloc[:, cc * L:(cc + 1) * L],
                        start=(cc == 0), stop=(cc == 1))
                sg = pscr.tile([128, L], bf16, tag="sg")
                nc.scalar.activation(sg[:], ps[:], AF.Sigmoid,
                                     bias=agb_t[:, mc:mc + 1])
                t = pscr.tile([128, L], f32, tag="gt")
                nc.vector.tensor_mul(t[:], sg[:], qTa[:, mc * L:(mc + 1) * L])
                nc.vector.tensor_add(xloc[:, mc * L:(mc + 1) * L],
                                     xloc[:, mc * L:(mc + 1) * L], t[:])

            # ln2 local -> hnloc
            ln_partition(xloc, L, 0, L, 2, alnw[:, 4:6], alnw[:, 6:8],
                         hnloc, L, 0, 1.0 / A)

            aff1_t = paw.tile([128, 2 * 1024], bf16, tag="aff1")
            for c in range(2):
                nc.sync.dma_start(aff1_t[:, c * 1024:(c + 1) * 1024],
                                  a_ff1[lyr, c])
            aff1b_t = paw.tile([128, 8], f32, tag="aff1b")
            nc.sync.dma_start(aff1b_t[:], a_ff1b[lyr])

            # ff1 + swiglu (a: chunks 0-3, g: 4-7)
            for j in range(4):
                a_ps = ppmm.tile([128, L], f32, tag="mm")
                for cc in range(2):
                    nc.tensor.matmul(
                        a_ps[:], aff1_t[:, cc * 1024 + j * 128: cc * 1024 + j * 128 + 128],
                        hnloc[:, cc * L:(cc + 1) * L],
                        start=(cc == 0), stop=(cc == 1))
                g_ps = ppmm.tile([128, L], f32, tag="mm")
                for cc in range(2):
                    nc.tensor.matmul(
                        g_ps[:], aff1_t[:, cc * 1024 + 512 + j * 128: cc * 1024 + 512 + j * 128 + 128],
                        hnloc[:, cc * L:(cc + 1) * L],
                        start=(cc == 0), stop=(cc == 1))
                sg = pscr.tile([128, L], bf16, tag="silu")
                nc.scalar.activation(sg[:], g_ps[:], AF.Silu,
                                     bias=aff1b_t[:, 4 + j:5 + j])
                av = pscr.tile([128, L], bf16, tag="av")
                nc.vector.tensor_scalar_add(av[:], a_ps[:],
                                            aff1b_t[:, j:j + 1])
                nc.vector.tensor_mul(swiA[:, j * L:(j + 1) * L], av[:], sg[:])

            aff2_t = paw.tile([128, 4 * A], bf16, tag="aff2")
            for c in range(4):
                nc.sync.dma_start(aff2_t[:, c * A:(c + 1) * A], a_ff2[lyr, c])
            aff2b_t = paw.tile([128, 2], f32, tag="aff2b")
            nc.sync.dma_start(aff2b_t[:], a_ff2b[lyr])

            for mc in range(2):
                ps = ppmm.tile([128, L], f32, tag="mm")
                for cc in range(4):
                    nc.tensor.matmul(
                        ps[:], aff2_t[:, cc * A + mc * 128: cc * A + mc * 128 + 128],
                        swiA[:, cc * L:(cc + 1) * L],
                        start=(cc == 0), stop=(cc == 3))
                t = pscr.tile([128, L], f32, tag="gt")
                nc.vector.tensor_scalar_add(t[:], ps[:],
                                            aff2b_t[:, mc:mc + 1])
                nc.vector.tensor_add(xloc[:, mc * L:(mc + 1) * L],
                                     xloc[:, mc * L:(mc + 1) * L], t[:])

            if lyr < ATOM_DEPTH - 1:
                atom_allgather()

        # final head: out_ln -> den^T
        oln_t = pconst.tile([128, 4], f32)
        nc.sync.dma_start(oln_t[:, 0:2], outln_d[0])
        nc.sync.dma_start(oln_t[:, 2:4], outln_d[1])
        outw_t = pconst.tile([128, 2 * 4], bf16)
        for c in range(2):
            nc.sync.dma_start(outw_t[:, c * 4:(c + 1) * 4], outw_d[c])
        ln_partition(xloc, L, 0, L, 2, oln_t[:, 0:2], oln_t[:, 2:4],
                     hnloc, L, 0, 1.0 / A)
        ps = ppmm.tile([128, L], f32, tag="mm")
        for cc in range(2):
            nc.tensor.matmul(ps[:4, :], outw_t[:, cc * 4:(cc + 1) * 4],
                             hnloc[:, cc * L:(cc + 1) * L],
                             start=(cc == 0), stop=(cc == 1))
        dsb = pscr.tile([4, L], f32, tag="den")
        nc.vector.tensor_copy(dsb[:], ps[:4, :])
        nc.sync.dma_start(denT_d[:], dsb[:])

        es.close()
    nc.compile()
    return nc


def _get_built():
    global _BUILT
    if _BUILT is None:
        _BUILT = _build()
    return _BUILT


# ---------------------------------------------------------------- host side
def _prep_core_inputs(x, aacodes, atcodes, aaindices, noised, nlev, params):
    """Returns list of 8 in_maps."""
    f32 = np.float32
    P = lambda t: np.asarray(t, dtype=f32)
    aacodes = np.asarray(aacodes).astype(np.int64)
    atcodes = np.asarray(atcodes).astype(np.int64)
    aaindices = np.asarray(aaindices).astype(np.int64)
    x = P(x); noised = P(noised); nlev = P(nlev)

    bias = _alibi_bias()                      # [H,L,L]
    s_seq = np.float32(DK ** -0.25)
    s_atom = np.float32((A // 8) ** -0.25)

    pos = _pos_encoding(aaindices, A)
    nl = {k: P(v) for k, v in params['nlev'].items()}
    four = _fourier_embed(nlev, nl)
    coordscale = np.sqrt(nlev[:, None, None] ** 2 + VARDATA)
    embed = (P(params['aa_embed'])[aacodes][aaindices][None]
             + P(params['atom_embed'])[atcodes][None]
             + pos[None] + four[:, None]
             + (noised / coordscale) @ P(params['coord_embed']))  # [B,NA,A]
    G = np.zeros((L, NA), f32)
    G[aaindices, np.arange(NA)] = 1.0

    x0 = _ln_np(x, P(params['norm1_g']), P(params['norm1_b']))    # [B,L,D]

    def bf(a):
        return np.ascontiguousarray(a).astype(BF16)

    def pcols(v, nch):      # feature vec [nch*128] -> [128, nch] f32
        return np.ascontiguousarray(np.asarray(v, f32).reshape(nch, 128).T)

    # ---- shared (per-layer) weights ----
    seq_w = dict(s_wqkv=[], s_wo=[], s_gw=[], s_gb=[], s_ln=[], s_ff1=[],
                 s_ff2=[])
    for p in params['seq']:
        seq_w['s_gb'].append(pcols(p['gb'], 8))
        seq_w['s_ln'].append(np.stack([pcols(p['ln1_g'], 8),
                                       pcols(p['ln1_b'], 8),
                                       pcols(p['ln2_g'], 8),
                                       pcols(p['ln2_b'], 8)]))
        seq_w['s_gw'].append(bf(P(p['gw']).reshape(8, 128, D)))
    atom_w = dict(a_wqkv=[], a_gw=[], a_gb=[], a_ln=[], a_ff1=[], a_ff1b=[],
                  a_ff2=[], a_ff2b=[])
    for p in params['atom']:
        w768 = np.concatenate([P(p['wq']) * s_atom, P(p['wk']) * s_atom,
                               P(p['wv'])], 1)
        atom_w['a_wqkv'].append(bf(w768.reshape(2, 128, 768)))
        atom_w['a_gw'].append(bf(P(p['gw']).reshape(2, 128, A)))
        atom_w['a_gb'].append(pcols(p['gb'], 2))
        atom_w['a_ln'].append(np.stack([pcols(p['ln1_g'], 2),
                                        pcols(p['ln1_b'], 2),
                                        pcols(p['ln2_g'], 2),
                                        pcols(p['ln2_b'], 2)]))
        atom_w['a_ff1'].append(bf(P(p['ff1']).reshape(2, 128, 1024)))
        atom_w['a_ff1b'].append(pcols(p['ff1_b'], 8))
        atom_w['a_ff2'].append(bf(P(p['ff2']).reshape(4, 128, A)))
        atom_w['a_ff2b'].append(pcols(p['ff2_b'], 2))
    shared = {k: np.stack(v) for k, v in {**seq_w, **atom_w}.items()
              if k not in ('s_wqkv', 's_wo', 's_ff1', 's_ff2')}
    shared['w4'] = np.ascontiguousarray(
        np.concatenate([P(params['to_coords']), P(params['to_confs'])],
                       1).reshape(8, 128, 4))
    shared['to_atom'] = np.ascontiguousarray(
        P(params['to_atom']).reshape(8, 128, A))
    shared['norm2'] = np.stack([P(params['norm2_g'])[None],
                                P(params['norm2_b'])[None]])
    shared['norm3'] = np.stack([pcols(params['norm3_g'], 2),
                                pcols(params['norm3_b'], 2)])
    shared['out_lnw'] = np.stack([pcols(params['out_ln_g'], 2),
                                  pcols(params['out_ln_b'], 2)])
    ow = np.concatenate([P(params['out_w']), np.zeros((A, 1), f32)], 1)
    shared['out_w'] = bf(ow.reshape(2, 128, 4))

    in_maps = []
    for c in range(N_CORES):
        b, tp = c // 4, c % 4
        m = dict(shared)
        m['xT0'] = np.ascontiguousarray(x0[b].T.reshape(8, 128, L))
        wqkv_l, wo_l, ff1_l, ff2_l = [], [], [], []
        for p in params['seq']:
            sl = slice(tp * 256, (tp + 1) * 256)
            w768 = np.concatenate([P(p['wq'])[:, sl] * s_seq,
                                   P(p['wk'])[:, sl] * s_seq,
                                   P(p['wv'])[:, sl]], 1)
            wqkv_l.append(bf(w768.reshape(8, 128, 768)))
            wo_l.append(bf(P(p['wo'])[sl].reshape(2, 128, D)))
            f1 = P(p['ff1'])
            a_w = f1[:, tp * 1024:(tp + 1) * 1024]
            g_w = f1[:, 4096 + tp * 1024:4096 + (tp + 1) * 1024]
            ff1_l.append(bf(np.concatenate([a_w, g_w], 1).reshape(8, 128, 2048)))
            ff2_l.append(bf(P(p['ff2'])[tp * 1024:(tp + 1) * 1024]
                            .reshape(8, 128, D)))
        m['s_wqkv'] = np.stack(wqkv_l)
        m['s_wo'] = np.stack(wo_l)
        m['s_ff1'] = np.stack(ff1_l)
        m['s_ff2'] = np.stack(ff2_l)
        eb = np.exp(bias[tp * 4:(tp + 1) * 4])          # [4,L,L] (q,kv)
        m['expb'] = bf(np.ascontiguousarray(
            eb.transpose(0, 2, 1).reshape(4, 4, 128, L)))
        m['g_loc'] = bf(np.ascontiguousarray(
            G[:, tp * L:(tp + 1) * L].reshape(4, 128, L)))
        m['embedT'] = np.ascontiguousarray(
            embed[b, tp * L:(tp + 1) * L].T.reshape(2, 128, L))
        in_maps.append(m)
    return in_maps, G


def kernel(x, aacodes, atcodes, aaindices, noised_coords_in, nlev_in, params):
    from concourse.bass_utils import run_bass_kernel_spmd
    nc = _get_built()
    in_maps, _ = _prep_core_inputs(x, aacodes, atcodes, aaindices,
                                   noised_coords_in, nlev_in, params)
    res = run_bass_kernel_spmd(nc, in_maps, core_ids=list(range(N_CORES)))
    outs = res.results

    noised = np.asarray(noised_coords_in, np.float32)
    nlev = np.asarray(nlev_in, np.float32)
    pred_coords = np.zeros((B, L, 3), np.float32)
    pred_confs = np.zeros((B, L), np.float32)
    den = np.zeros((B, NA, 3), np.float32)
    for b in range(B):
        c4 = outs[b * 4]['coords4'].reshape(L, 4)
        pred_coords[b] = c4[:, :3]
        pred_confs[b] = c4[:, 3]
        for tp in range(4):
            dT = outs[b * 4 + tp]['denT']        # [4, L]
            den[b, tp * L:(tp + 1) * L] = dT[:3].T
    t_h = nlev[:, None, None]
    pred_denoised = (den * SIGDATA * t_h / np.sqrt(VARDATA + t_h ** 2)
                     + noised * VARDATA / (VARDATA + t_h ** 2))
    return pred_denoised, pred_coords, pred_confs
